# revision 2
# baseline (speedup 1.0000x reference)
"""Trainium2 Bass kernel for nn_AttentionS2 (spherical self-attention).

Module: y = p_w @ softmax_k(q k^T / sqrt(hd) + log_quad_w[k]) v + p_b
with q/k/v = 1x1-conv projections of the same input (self-attention),
B=1, C=512, H=W=64 (4096 tokens), 8 heads, head_dim=64.

Sharding: one head per NeuronCore (8 cores).

Key structure (per core):
  * All matmul operands are bf16 (x and weights are converted host-side),
    so weight loads use FWL (4x) and moving operands stream at full rate.
  * The additive log-quadrature bias is folded multiplicatively into v:
    exp(s*S + lqw_k) = qw_k * exp(s*S), with qw_k also replacing the ones
    column used for the softmax denominator.  The exp is therefore
    bias-free and an exp tile can be any (key-tile x query-span) block.
  * k and q projections share one combined stationary [wk | A*wq] so the
    projection runs at M=128 (full PE columns).  A = 128*log2(e)/8 is a
    Schraudolph pre-scale folded into wq: the S matmul then directly
    produces logits in bf16-pattern units.
  * S^T is computed in (key x query) orientation with K=64 row tiling:
    two concurrent 64-row PE tiles (auto tile_position from base
    partitions) double S throughput.
  * exp is split across TWO engines: ACT (table exp, out bf16) and a
    custom DVE op (corrected Schraudolph: int16 round-to-nearest of
    y - k*(128|a|-a^2)/128 where y = x + B yields the bf16 bit pattern
    of exp; max rel err ~0.7%).  The split fraction is compile-time.
  * AV accumulates [v'|qw]^T P in PSUM over 32 key tiles; normalization
    uses a fast approximate reciprocal + a K=1 ones-matmul broadcast.
  * AllToAll reshards head-major outputs (bf16) to token-major chunks;
    each core applies the output projection on its 512-token slice.
"""

import contextlib
import os
import sys
import types

import numpy as np
import ml_dtypes

import concourse.bass as bass
import concourse.bacc as bacc
import concourse.tile as tile
from concourse import mybir
from concourse import bass_utils

# This container has no axon NTFF profile hook; shim the module so
# run_bass_kernel_spmd(trace=True) degrades gracefully instead of raising.
try:  # pragma: no cover
    import antenv.axon_hooks  # noqa: F401
except Exception:  # ModuleNotFoundError, or antenv missing entirely
    try:
        import antenv  # noqa: F401
    except Exception:
        antenv_mod = types.ModuleType("antenv")
        sys.modules["antenv"] = antenv_mod
    shim = types.ModuleType("antenv.axon_hooks")
    shim.get_axon_ntff_profile_hook = lambda: None
    sys.modules["antenv.axon_hooks"] = shim

F32 = mybir.dt.float32
F32R = mybir.dt.float32r
BF16 = mybir.dt.bfloat16
I16 = mybir.dt.int16
AF = mybir.ActivationFunctionType

C = 512          # channels
T = 4096         # tokens (H*W)
HD = 64          # head dim
NCORES = 8
NKT = T // 128   # 32 key tiles of 128
QC = 1024        # query chunk width for the attention inner loop
NQC = T // QC    # 4
CT = T // NCORES  # 512 tokens per core in the output projection
SCALE = 1.0 / float(np.sqrt(HD))

# Schraudolph constants: logits arrive pre-scaled by A (folded into wq),
# i.e. psum = A * (q.k) with A = 128*log2(e)*SCALE.  Then the bf16 bit
# pattern of exp(SCALE*q.k) is round(y + corr), y = psum + B.
A_PRE = float(128.0 * np.log2(np.e) * SCALE)
B_SCH = 16255.8
K_SCH = 0.335
C0_SCH = float(np.float32(B_SCH + 3.0 * 2.0**29))
ACT_SCALE = float(np.log(2.0) / 128.0)   # ACT exp: e^(ACT_SCALE * psum)

_CACHE = {}

# exp-engine split: step g -> DVE iff pattern[g % len] set.  qc==0 uses a
# sparser pattern (DVE also does projection copies there).
import os as _os
_PATSEL = _os.environ.get("KERNEL_DVE_PAT", "std")
if _PATSEL == "none" or _os.environ.get("KERNEL_NO_DVE_EXP", "0") == "1":
    DVE_PAT = (0,)
    DVE_PAT0 = (0,)
elif _PATSEL == "alt":
    DVE_PAT = (0, 1)
    DVE_PAT0 = (0, 1)
elif _PATSEL == "all":
    DVE_PAT = (1,)
    DVE_PAT0 = (1,)
else:
    DVE_PAT = (0, 1, 0, 1, 0, 0, 1, 0, 1, 0, 1, 0)   # 5/12 ~= 0.42
    DVE_PAT0 = (0, 0, 1, 0)                          # 1/4 for qc==0
_VARIANT = "notail" if _os.environ.get("KERNEL_NOTAIL", "0") == "1" else "full"



def _register_exp_op():
    """Register the corrected-Schraudolph exp custom DVE op (idempotent)."""
    from concourse import dve_ops as dvo
    from concourse.dve_spec import Spec, Src0, Src1, C0, C1, C2, lower, Bin, AluOp

    name = "SCHRAUDOLPH_EXP_BF16_ANT"
    for op in dvo.OPS:
        if op.name == name:
            return op
    # y = x + B; u = x + (B + 3*2^29) rounds to the 128 grid; v = u - 3*2^29
    # b = |y - v|; corr = b*(b-128)*(k/128);  out = y + corr  -> int16 RN
    y = Src0 + C1
    u = Src0 + C0
    v = u - (C0 - C1)
    b = Bin(AluOp.ABSOLUTE_DIFF, y, v)
    t = b * (b - C2)
    spec = Spec(body=y + t * Src1)
    row = dvo._CUSTOM_DVE_ROW_BASE + len(dvo.OPS)
    assert row < 0x20
    dvo._SUB_OPCODE_FOR_NAME[name] = row
    shas = {}
    for ver in ("v3", "v4"):
        compiled = bass_utils.DveOpSpec(
            name=name, opcode=row, uops=lower(spec, ver=ver), rd1_en=True)
        shas[ver] = compiled.sha(ver)
    op = dvo.DveOp(name, spec, subdim=False, uops_sha=shas)
    dvo.OPS.append(op)
    dvo.CUSTOM_DVE_SPECS[name] = spec
    return op


EXP_OP = _register_exp_op()


def _dve_steps():
    s = set()
    for qc in range(NQC):
        pat = DVE_PAT0 if qc == 0 else DVE_PAT
        for kt in range(NKT):
            g = qc * NKT + kt
            if pat[g % len(pat)]:
                s.add(g)
    return s


def _emit_body(nc, tc, io, rep):
    """Emit one full forward pass. `io` holds the DRAM tensor handles.

    Emission order software-pipelines the attention inner loop: the S^T
    matmuls run two iterations ahead of exp/AV.  Projections are
    interleaved into the qc==0 attention iterations so the first exp can
    start early while the rest of x is still loading.
    """
    x, wqk, wv, wp, bqk, bv, pb, qwf, qwb, onesr, ksch, y, dbg = io
    dve_set = _dve_steps()
    with contextlib.ExitStack() as ctx:
        big = ctx.enter_context(tc.tile_pool(name=f"big{rep}", bufs=1))
        wts = ctx.enter_context(tc.tile_pool(name=f"wts{rep}", bufs=1))
        vtp = ctx.enter_context(tc.tile_pool(name=f"vtp{rep}", bufs=1))
        ptlp = ctx.enter_context(tc.tile_pool(name=f"ptl{rep}", bufs=8))
        sml = ctx.enter_context(tc.tile_pool(name=f"sml{rep}", bufs=6))
        drp = ctx.enter_context(tc.tile_pool(name=f"drp{rep}", bufs=1, space="DRAM"))

        ps_stack = contextlib.ExitStack()
        # shared PSUM pool for projection + S staging (3 x 2 banks)
        # plus the AV accumulator / reciprocal broadcast (2 banks) = 8.
        pss = ps_stack.enter_context(
            tc.tile_pool(name=f"pss{rep}", bufs=3, space="PSUM"))
        psa = ps_stack.enter_context(
            tc.tile_pool(name=f"psa{rep}", bufs=1, space="PSUM"))

        # ---- weight/const loads ---------------------------------------
        wqk_sb = wts.tile([128, 4, 128], BF16, tag="wqk")
        wv_sb = wts.tile([128, 4, HD], BF16, tag="wv")
        wp_sb = wts.tile([128, 4, C], BF16, tag="wp")
        for ci in range(4):
            cs = slice(128 * ci, 128 * (ci + 1))
            nc.sync.dma_start(out=wqk_sb[:, ci, :], in_=wqk[cs, :])
            nc.sync.dma_start(out=wv_sb[:, ci, :], in_=wv[cs, :])
            nc.sync.dma_start(out=wp_sb[:, ci, :], in_=wp[cs, :])
        bqk_sb = wts.tile([128, 1], F32, tag="bqk")
        bv_sb = wts.tile([HD, 1], F32, tag="bv")
        pb_sb = wts.tile([128, 4], F32, tag="pb")
        qwf_sb = wts.tile([128, NKT], F32, tag="qwf")
        nc.sync.dma_start(out=bqk_sb, in_=bqk[:, :])
        nc.sync.dma_start(out=bv_sb, in_=bv[:, :])
        nc.sync.dma_start(out=pb_sb, in_=pb[:, :])
        nc.sync.dma_start(out=qwf_sb, in_=qwf[:, :])
        onesr_sb = wts.tile([1, HD], F32R, tag="onesr")
        nc.sync.dma_start(out=onesr_sb, in_=onesr[:, :])
        # full-size Src1 constant: [P,1]-broadcast Src1 crashes the DVE on
        # this silicon/runtime, so the k/128 constant is a full-width tile.
        ksch_sb = wts.tile([128, QC], F32, tag="ksch")
        nc.sync.dma_start(out=ksch_sb, in_=ksch[:, :])

        # ---- x loads, in 512-token groups so compute starts early -----
        x_sb = big.tile([128, 4, T], BF16, tag="x")

        def load_x_group(g):
            for ci in range(4):
                nc.sync.dma_start(
                    out=x_sb[:, ci, 512 * g:512 * (g + 1)],
                    in_=x[128 * ci:128 * (ci + 1), 512 * g:512 * (g + 1)])

        # qk_sb: k on rows 0:64 (S sub0 stationary), A*q on rows 64:128
        # (S sub1 moving).  kq2: the swapped copy (q rows 0:64, k 64:128).
        qk_sb = big.tile([128, T], BF16, tag="qk")
        kq2 = big.tile([128, T], BF16, tag="kq2")
        vt = []
        for t in range(NKT):
            vt_t = vtp.tile([128, HD + 1], BF16, tag=f"vt{t}")
            vt.append(vt_t)

        def emit_qk_chunk(n):
            sl = slice(512 * n, 512 * (n + 1))
            ps = pss.tile([128, 512], F32, tag="ss")
            for ci in range(4):
                nc.tensor.matmul(ps, wqk_sb[:, ci, :], x_sb[:, ci, sl],
                                 start=(ci == 0), stop=(ci == 3))
            nc.vector.tensor_scalar_add(out=qk_sb[:, sl], in0=ps,
                                        scalar1=bqk_sb)
            nc.sync.dma_start(out=kq2[0:HD, sl], in_=qk_sb[HD:128, sl])
            nc.sync.dma_start(out=kq2[HD:128, sl], in_=qk_sb[0:HD, sl])
            if dbg is not None and n == 0:
                nc.sync.dma_start(out=dbg["qk0"][:, :], in_=qk_sb[:, 0:512])

        def emit_vt(t):
            # token-major v' tile: qw-scaled v plus the qw column (denom)
            ps = pss.tile([128, 512], F32, tag="ss")
            for ci in range(4):
                nc.tensor.matmul(ps[:, 0:HD],
                                 x_sb[:, ci, 128 * t:128 * (t + 1)],
                                 wv_sb[:, ci, :],
                                 start=(ci == 0), stop=(ci == 3))
            nc.vector.tensor_scalar_mul(out=vt[t][:, 0:HD], in0=ps[:, 0:HD],
                                        scalar1=qwf_sb[:, t:t + 1])
            nc.sync.dma_start(out=vt[t][:, HD:HD + 1], in_=qwb[:, t:t + 1])
            if dbg is not None and t == 0:
                nc.sync.dma_start(out=dbg["vt0"][:, :], in_=vt[t][:, :])

        load_x_group(0)
        load_x_group(1)
        load_x_group(2)
        emit_qk_chunk(0)
        emit_qk_chunk(1)
        for t in range(8):
            emit_vt(t)

        # ---- attention (flat software pipeline over (qc, kt)) ----------
        oh = None  # per-qc output tiles come from sml pool
        snd = drp.tile([NCORES, HD, CT], BF16, tag="snd")
        rcv = drp.tile([NCORES, HD, CT], BF16, tag="rcv")

        ss_tiles = {}

        def emit_s(qc, kt):
            ss = pss.tile([128, QC], F32, tag="ss")
            ss_tiles[(qc, kt)] = ss
            ks = slice(128 * kt, 128 * (kt + 1))
            q0 = QC * qc
            # sub0: rows 0:64 (k stationary from qk_sb, q moving from kq2)
            nc.tensor.matmul(ss[:, 0:512],
                             qk_sb[0:HD, ks], kq2[0:HD, q0:q0 + 512],
                             start=True, stop=True)
            # sub1: rows 64:128 (k stationary from kq2, q moving from qk_sb)
            nc.tensor.matmul(ss[:, 512:QC],
                             kq2[HD:128, ks], qk_sb[HD:128, q0 + 512:q0 + QC],
                             start=True, stop=True)

        # interleaved projection work during qc==0, keyed by step kt.
        prefetch = {}
        for cn in range(2, 8):
            kt0 = 4 * (cn - 2)
            prefetch.setdefault(kt0, []).append(("qk", cn))
            for t in range(4 * cn, 4 * cn + 4):
                prefetch.setdefault(kt0 + 1, []).append(("vt", t))
            if cn + 1 < 8:
                prefetch.setdefault(kt0, []).append(("xg", cn + 1))

        steps = [(qc, kt) for qc in range(NQC) for kt in range(NKT)]
        av_tiles = {}
        emit_s(*steps[0])
        emit_s(*steps[1])
        for g, (qc, kt) in enumerate(steps):
            if qc == 0:
                for item in prefetch.get(kt, ()):
                    if item[0] == "xg":
                        load_x_group(item[1])
                    elif item[0] == "qk":
                        emit_qk_chunk(item[1])
                    elif item[0] == "vt":
                        emit_vt(item[1])
            if kt == 0:
                av_tiles[qc] = psa.tile([HD + 1, QC], F32, tag="av",
                                        name=f"av{qc}")
            av = av_tiles[qc]
            ss = ss_tiles.pop((qc, kt))
            pt = ptlp.tile([128, QC], I16, tag="pt")
            if g in dve_set:
                nc.vector._custom_dve(EXP_OP, out=pt, in0=ss,
                                      in1=ksch_sb, s0=C0_SCH, s1=B_SCH,
                                      imm2=128.0)
            else:
                nc.scalar.activation(out=pt.bitcast(BF16), in_=ss,
                                     func=AF.Exp, scale=ACT_SCALE)
            if g + 2 < len(steps):
                emit_s(*steps[g + 2])
            if dbg is not None and g == 0:
                nc.sync.dma_start(out=dbg["pt0"][:, :], in_=pt[:, :])
            ptb = pt.bitcast(BF16)
            for sub in range(2):
                nc.tensor.matmul(av[:, 512 * sub:512 * (sub + 1)],
                                 vt[kt], ptb[:, 512 * sub:512 * (sub + 1)],
                                 start=(kt == 0), stop=(kt == NKT - 1),
                                 skip_group_check=True)
            if kt == NKT - 1:
                # copy the accumulator out of PSUM so the bank frees, then
                # normalize: rows 0:64 numerator, row 64 denominator.  The
                # denominator is copied to a base-partition-0 tile: custom
                # DVE ops (reciprocal_approx_fast) misread partition-offset
                # inputs on this silicon/runtime.
                av_sb = sml.tile([HD, QC], F32, tag="avs")
                nc.vector.tensor_copy(out=av_sb, in_=av[0:HD, :])
                den_t = sml.tile([1, QC], F32, tag="den")
                nc.scalar.copy(out=den_t, in_=av[HD:HD + 1, :])
                rcp = sml.tile([1, QC], F32, tag="rcp")
                with nc.allow_low_precision(reason="~18-bit 1/den for softmax"):
                    nc.vector.reciprocal_approx_fast(out=rcp, in_=den_t)
                rcr = sml.tile([1, QC], F32R, tag="rcr")
                nc.vector.tensor_copy(out=rcr, in_=rcp)
                rb = psa.tile([HD, QC], F32, tag="av", name=f"rb{qc}")
                for sub in range(2):
                    nc.tensor.matmul(rb[:, 512 * sub:512 * (sub + 1)],
                                     onesr_sb,
                                     rcr[:, 512 * sub:512 * (sub + 1)],
                                     start=True, stop=True)
                if dbg is not None and qc == 0:
                    nc.sync.dma_start(out=dbg["av0"][0:HD, :], in_=av_sb[:, :])
                    nc.sync.dma_start(out=dbg["av0"][HD:HD + 1, :], in_=den_t[:, :])
                ohp = sml.tile([HD, QC], BF16, tag="oh")
                nc.vector.tensor_mul(out=ohp, in0=av_sb, in1=rb)
                nc.vector.tensor_scalar_add(out=ohp, in0=ohp, scalar1=bv_sb)
                if dbg is not None and qc == 0:
                    nc.sync.dma_start(out=dbg["oh0"][:, :], in_=ohp[:, :])
                    nc.sync.dma_start(out=dbg["rcp0"][:, :], in_=rcp[:, :])
                    nc.sync.dma_start(out=dbg["rcr0"][:, :], in_=rcr[:, :])
                for d2 in range(QC // CT):
                    dest = (QC // CT) * qc + d2
                    so = CT * d2
                    nc.sync.dma_start(out=snd[dest, :, :],
                                      in_=ohp[:, so:so + CT])

        # ---- reshard head-major -> token-major, then output projection -
        if _VARIANT == "notail":
            # sim-only: stand in for the AllToAll with a local DRAM copy so
            # TimelineSim (no collectives) can model the full program.
            nc.sync.dma_start(out=rcv[:, :, :], in_=snd[:, :, :])
        else:
            nc.gpsimd.collective_compute(
                "AllToAll", mybir.AluOpType.bypass,
                replica_groups=[list(range(NCORES))],
                ins=[snd[:, :, :]], outs=[rcv[:, :, :]])
        at = big.tile([128, 4, CT], BF16, tag="at")
        rcv_flat = rcv[:, :, :].rearrange("a b c -> (a b) c")
        for ci in range(4):
            nc.sync.dma_start(out=at[:, ci, :],
                              in_=rcv_flat[128 * ci:128 * (ci + 1), :])
        ps_stack.close()
        with tc.tile_pool(name=f"psy{rep}", bufs=2, space="PSUM") as psy:
            for m in range(4):
                ps = psy.tile([128, CT], F32, tag="yps")
                for ci in range(4):
                    nc.tensor.matmul(ps, wp_sb[:, ci, 128 * m:128 * (m + 1)],
                                     at[:, ci, :],
                                     start=(ci == 0), stop=(ci == 3))
                yo = sml.tile([128, CT], F32, tag="yo")
                nc.vector.tensor_scalar_add(out=yo, in0=ps,
                                            scalar1=pb_sb[:, m:m + 1])
                nc.sync.dma_start(out=y[128 * m:128 * (m + 1), :], in_=yo)


def _build(repeat=1):
    nc = bacc.Bacc("TRN2", target_bir_lowering=False, debug=False,
                   num_devices=NCORES)
    x = nc.dram_tensor("x", [C, T], BF16, kind="ExternalInput")
    wqk = nc.dram_tensor("wqk", [C, 128], BF16, kind="ExternalInput")
    wv = nc.dram_tensor("wv", [C, HD], BF16, kind="ExternalInput")
    wp = nc.dram_tensor("wp", [C, C], BF16, kind="ExternalInput")
    bqk = nc.dram_tensor("bqk", [128, 1], F32, kind="ExternalInput")
    bv = nc.dram_tensor("bv", [HD, 1], F32, kind="ExternalInput")
    pb = nc.dram_tensor("pb", [128, 4], F32, kind="ExternalInput")
    qwf = nc.dram_tensor("qwf", [128, NKT], F32, kind="ExternalInput")
    qwb = nc.dram_tensor("qwb", [128, NKT], BF16, kind="ExternalInput")
    onesr = nc.dram_tensor("onesr", [1, HD], F32R, kind="ExternalInput")
    ksch = nc.dram_tensor("ksch", [128, QC], F32, kind="ExternalInput")
    y = nc.dram_tensor("y", [C, CT], F32, kind="ExternalOutput")
    dbg = None
    if os.environ.get("KERNEL_DEBUG", "0") == "1":
        dbg = dict(
            qk0=nc.dram_tensor("dbg_qk0", [128, 512], BF16, kind="ExternalOutput"),
            vt0=nc.dram_tensor("dbg_vt0", [128, HD + 1], BF16, kind="ExternalOutput"),
            pt0=nc.dram_tensor("dbg_pt0", [128, QC], I16, kind="ExternalOutput"),
            av0=nc.dram_tensor("dbg_av0", [HD + 1, QC], F32, kind="ExternalOutput"),
            oh0=nc.dram_tensor("dbg_oh0", [HD, QC], BF16, kind="ExternalOutput"),
            rcp0=nc.dram_tensor("dbg_rcp0", [1, QC], F32, kind="ExternalOutput"),
            rcr0=nc.dram_tensor("dbg_rcr0", [1, QC], F32R, kind="ExternalOutput"),
        )
    io = (x, wqk, wv, wp, bqk, bv, pb, qwf, qwb, onesr, ksch, y, dbg)

    with tile.TileContext(nc) as tc:
        for rep in range(repeat):
            _emit_body(nc, tc, io, rep)

    nc.finalize()
    return nc


def _get_nc(repeat=1):
    key = ("nc", repeat)
    if key not in _CACHE:
        _CACHE[key] = _build(repeat)
    return _CACHE[key]


def _quad_weights():
    # Clenshaw-Curtis quadrature weights on the 64-lat equiangular grid,
    # as torch-harmonics' 'equiangular' grid: flipped, * 2*pi/nlon.
    n = 64
    N = n - 1
    theta = np.pi * np.arange(n) / N
    m = N // 2
    j = np.arange(1, m + 1)
    b = np.where(2 * j == N, 1.0, 2.0)
    S = (b / (4.0 * j**2 - 1.0))[None, :] * np.cos(
        2.0 * j[None, :] * theta[:, None])
    w = 1.0 - S.sum(axis=1)
    c = np.full(n, 2.0)
    c[0] = 1.0
    c[-1] = 1.0
    w = (c * w / N)[::-1].copy()
    qw = 2.0 * np.pi * w / 64.0                       # (nlat,)
    return np.tile(qw[:, None], (1, 64)).reshape(-1)  # (T,)


def _in_maps(query, q_w, q_b, k_w, k_b, v_w, v_b, p_w, p_b, log_quad_weights):
    bf = ml_dtypes.bfloat16
    x = np.ascontiguousarray(
        np.asarray(query, np.float32).reshape(C, T)).astype(bf)
    wp = np.ascontiguousarray(np.asarray(p_w, np.float32).T).astype(bf)
    pbm = np.ascontiguousarray(np.asarray(p_b, np.float32).reshape(4, 128).T)
    qw = np.exp(np.asarray(log_quad_weights, np.float32)).astype(np.float64)
    qwf = np.ascontiguousarray(qw.reshape(NKT, 128).T.astype(np.float32))
    qwb = np.ascontiguousarray(qwf.astype(bf))
    maps = []
    for h in range(NCORES):
        hs = slice(HD * h, HD * (h + 1))
        wk_h = np.asarray(k_w, np.float32)[hs, :].T          # [C, 64]
        wq_h = np.asarray(q_w, np.float32)[hs, :].T * A_PRE  # [C, 64]
        wqk = np.ascontiguousarray(
            np.concatenate([wk_h, wq_h], axis=1)).astype(bf)
        bqk = np.concatenate([
            np.asarray(k_b, np.float32)[hs],
            np.asarray(q_b, np.float32)[hs] * A_PRE]).reshape(128, 1)
        maps.append(dict(
            x=x,
            wqk=wqk,
            wv=np.ascontiguousarray(
                np.asarray(v_w, np.float32)[hs, :].T).astype(bf),
            wp=wp,
            bqk=np.ascontiguousarray(bqk),
            bv=np.ascontiguousarray(
                np.asarray(v_b, np.float32)[hs].reshape(HD, 1)),
            pb=pbm,
            qwf=qwf,
            qwb=qwb,
            onesr=np.ones((1, HD), np.float32),
            ksch=np.full((128, QC), K_SCH / 128.0, np.float32),
        ))
    return maps


def _run(in_maps, repeat=1, **kw):
    nc = _get_nc(repeat)
    return bass_utils.run_bass_kernel_spmd(nc, in_maps, list(range(NCORES)), **kw)


def _assemble(results):
    # core c owns output tokens [CT*c, CT*(c+1))
    full = np.concatenate([results[c]["y"] for c in range(NCORES)], axis=1)
    return np.ascontiguousarray(full.reshape(1, C, 64, 64).astype(np.float32))


def kernel(**inputs):
    res = _run(_in_maps(**inputs))
    return _assemble(res.results)

